# revision 10
# baseline (speedup 1.0000x reference)
"""Trainium2 Bass kernel for BestOfKSoftminOT.

Math per (b, k) pair:
  X = sim_seq[b]            [T, d]
  Y = expert[b, s:s+T]      [T, d]
  C[i,j] = max(|x_i|^2 + |y_j|^2 - 2 x_i.y_j, 0)
  log-domain Sinkhorn, 60 iters, eps=0.1; Lk = sum(P*C)
  loss = -tau * mean_b logsumexp_k(-Lk/tau)

Device algorithm (per pair, per core; 16 pairs per core, 8 cores):
  Mneg = relu((xx + yy - 2 x.y)/eps) = C/eps    (augmented 34-dim matmul on PE)
  stored twice: row-major tiles ([i=part, j=free]) and transposed.
  Per half-iteration (row-update shown):
    tmp  = Mneg - bv            (DVE tensor_tensor_reduce, accum=rowmin -> mm)
    e    = exp(-tmp + mm)       (ACT, bias=mm per partition, accum=rowsum -> s)
    g    = mm - ln(T*s)         (ACT Ln + DVE sub)  == log_a - lse
    bv'  = broadcast(g)         (PE transpose [128,4]->[4,128], evac, 4 bcast matmuls)
  Final: P = exp(-(Mneg - bv_v) + gu);  pc = eps * rowsum(P*Mneg);
         lk_tile[1,4] = ones^T @ pc;  DMA out.
Host: gathers crops, builds augmented operands, sums 4 partials per pair,
      then softmin-over-K mean in fp64.
"""

import sys
from contextlib import ExitStack

import numpy as np

sys.path.insert(0, "/opt/trn_rl_repo")

import concourse.bass as bass
import concourse.bacc as bacc
import concourse.tile as tile
from concourse import mybir
from concourse.masks import make_identity
from concourse.bass_utils import run_bass_kernel_spmd

B, T, K, D = 16, 512, 8, 32
EPS, ITERS, TAU = 0.1, 60, 0.5
NCORES = 8
PAIRS = B * K // NCORES  # 16 pairs per core
NT = T // 128  # 4 partition tiles
F32 = mybir.dt.float32
BIG = 3.0e38
ALU = mybir.AluOpType
AF = mybir.ActivationFunctionType


def build_program(pairs=PAIRS, iters=ITERS, unroll=10):
    nc = bacc.Bacc("TRN2")
    xa_d = nc.declare_dram_parameter("xa", [pairs, 34, 512], F32, isOutput=False)
    ya_d = nc.declare_dram_parameter("ya", [pairs, 34, 512], F32, isOutput=False)
    xb_d = nc.declare_dram_parameter("xb", [pairs, 34, 512], F32, isOutput=False)
    yb_d = nc.declare_dram_parameter("yb", [pairs, 34, 512], F32, isOutput=False)
    out_d = nc.declare_dram_parameter("out", [pairs, NT], F32, isOutput=True)

    with tile.TileContext(nc) as tc, ExitStack() as ctx:
        consts = ctx.enter_context(tc.tile_pool(name="consts", bufs=1))
        inpool = ctx.enter_context(tc.tile_pool(name="inp", bufs=2))
        mats = ctx.enter_context(tc.tile_pool(name="mats", bufs=2))
        work = ctx.enter_context(tc.tile_pool(name="work", bufs=2))
        small = ctx.enter_context(tc.tile_pool(name="small", bufs=2))
        ps_bv = ctx.enter_context(tc.tile_pool(name="psbv", bufs=2, space="PSUM"))
        ps_mm = ctx.enter_context(tc.tile_pool(name="psmm", bufs=2, space="PSUM"))
        ps_sm = ctx.enter_context(tc.tile_pool(name="pssm", bufs=2, space="PSUM"))

        ident = consts.tile([128, 128], F32)
        make_identity(nc, ident)
        ones128 = consts.tile([128, 1], F32)
        nc.vector.memset(ones128, 1.0)

        for p in range(pairs):
            xa_sb = inpool.tile([34, 512], F32, tag="xa")
            ya_sb = inpool.tile([34, 512], F32, tag="ya")
            xb_sb = inpool.tile([34, 512], F32, tag="xb")
            yb_sb = inpool.tile([34, 512], F32, tag="yb")
            nc.sync.dma_start(out=xa_sb[:], in_=xa_d[p])
            nc.sync.dma_start(out=ya_sb[:], in_=ya_d[p])
            nc.sync.dma_start(out=xb_sb[:], in_=xb_d[p])
            nc.sync.dma_start(out=yb_sb[:], in_=yb_d[p])

            Mneg = mats.tile([128, NT, 512], F32, tag="M")
            MnegT = mats.tile([128, NT, 512], F32, tag="MT")
            for t in range(NT):
                mm = ps_mm.tile([128, 512], F32, tag="mm")
                nc.tensor.matmul(mm[:], xa_sb[:, t * 128 : (t + 1) * 128], ya_sb[:])
                nc.scalar.activation(Mneg[:, t, :], mm[:], AF.Relu, scale=-1.0)
            for t in range(NT):
                mm = ps_mm.tile([128, 512], F32, tag="mm")
                nc.tensor.matmul(mm[:], yb_sb[:, t * 128 : (t + 1) * 128], xb_sb[:])
                nc.scalar.activation(MnegT[:, t, :], mm[:], AF.Relu, scale=-1.0)

            tmp = work.tile([128, NT, 512], F32, tag="tmp")
            e_scr = work.tile([128, 512], F32, tag="escr")
            mm_u = small.tile([128, NT], F32, tag="mmu")
            s_u = small.tile([128, NT], F32, tag="su")
            L_u = small.tile([128, NT], F32, tag="Lu")
            gu = small.tile([128, NT], F32, tag="gu")
            mm_v = small.tile([128, NT], F32, tag="mmv")
            s_v = small.tile([128, NT], F32, tag="sv")
            L_v = small.tile([128, NT], F32, tag="Lv")
            gv = small.tile([128, NT], F32, tag="gv")
            pc = small.tile([128, NT], F32, tag="pc")

            bv_u = ps_bv.tile([128, 512], F32, tag="bvu")
            bv_v = ps_bv.tile([128, 512], F32, tag="bvv")
            lk = ps_sm.tile([1, NT], F32, tag="lk")

            nc.vector.memset(bv_v[:], 0.0)

            def half(Msrc, bv_in, mm_st, s_st, L_st, g_st, bv_out):
                for t in range(NT):
                    nc.vector.tensor_tensor(
                        out=tmp[:, t, :],
                        in0=Msrc[:, t, :],
                        in1=bv_in[:],
                        op=ALU.subtract,
                    )
                    nc.vector.tensor_reduce(
                        out=mm_st[:, t : t + 1],
                        in_=tmp[:, t, :],
                        axis=mybir.AxisListType.X,
                        op=ALU.min,
                    )
                    nc.scalar.activation(
                        e_scr[:],
                        tmp[:, t, :],
                        AF.Exp,
                        bias=mm_st[:, t : t + 1],
                        scale=-1.0,
                        accum_out=s_st[:, t : t + 1],
                    )
                nc.scalar.activation(L_st[:], s_st[:], AF.Ln, scale=float(T))
                nc.vector.tensor_sub(g_st[:], mm_st[:], L_st[:])
                # broadcast: bv_out[q, 128t+c] = g_st[c, t] via
                # matmul(lhsT=g_st[:, t] bcast along free, rhs=identity)
                for t in range(NT):
                    nc.tensor.matmul(
                        bv_out[:, t * 128 : (t + 1) * 128],
                        g_st[:, t : t + 1].to_broadcast([128, 128]),
                        ident[:],
                    )

            def one_iter():
                half(Mneg, bv_v, mm_u, s_u, L_u, gu, bv_u)
                half(MnegT, bv_u, mm_v, s_v, L_v, gv, bv_v)

            n_chunks = iters // unroll
            rem = iters - n_chunks * unroll
            if n_chunks > 1:
                with tc.For_i(0, n_chunks, 1):
                    for _ in range(unroll):
                        one_iter()
            else:
                for _ in range(n_chunks * unroll):
                    one_iter()
            for _ in range(rem):
                one_iter()

            # final: P = exp(-(Mneg - bv_v) + gu); pc = eps * rowsum(P * Mneg)
            for t in range(NT):
                e_t = work.tile([128, 512], F32, tag="efin")
                nc.vector.tensor_tensor(
                    out=tmp[:, t, :], in0=Mneg[:, t, :], in1=bv_v[:], op=ALU.subtract
                )
                nc.scalar.activation(
                    e_t[:], tmp[:, t, :], AF.Exp, bias=gu[:, t : t + 1], scale=-1.0
                )
                nc.vector.tensor_tensor(
                    out=tmp[:, t, :], in0=e_t[:], in1=Mneg[:, t, :], op=ALU.mult
                )
                nc.vector.tensor_reduce(
                    out=pc[:, t : t + 1],
                    in_=tmp[:, t, :],
                    axis=mybir.AxisListType.X,
                    op=ALU.add,
                )
            nc.tensor.matmul(lk[:], ones128[:], pc[:])
            lk_sb = small.tile([1, NT], F32, tag="lksb")
            nc.vector.tensor_copy(lk_sb[:], lk[:])
            nc.sync.dma_start(out=out_d[p], in_=lk_sb[:])

    nc.compile()
    return nc


def host_prep(sim_seq, expert, starts):
    """Build per-core augmented matmul operands.

    Core c handles global pairs g = c*PAIRS + p, with b = g // K, k = g % K.
    """
    sim_seq = np.asarray(sim_seq, dtype=np.float32)
    expert = np.asarray(expert, dtype=np.float32)
    starts = np.asarray(starts).astype(np.int64)

    in_maps = []
    for c in range(NCORES):
        xa = np.empty((PAIRS, 34, 512), dtype=np.float32)
        ya = np.empty((PAIRS, 34, 512), dtype=np.float32)
        xb = np.empty((PAIRS, 34, 512), dtype=np.float32)
        yb = np.empty((PAIRS, 34, 512), dtype=np.float32)
        for p in range(PAIRS):
            g = c * PAIRS + p
            b, k = g // K, g % K
            s = int(starts[b, k])
            X = sim_seq[b]  # [T, d]
            Y = expert[b, s : s + T]  # [T, d]
            xx = (X * X).sum(-1)
            yy = (Y * Y).sum(-1)
            # M_raw[i,j] = (2 x.y - xx - yy)/eps ; Mneg = relu(-M_raw)
            xa[p, :D] = X.T
            xa[p, D] = xx
            xa[p, D + 1] = 1.0
            ya[p, :D] = (2.0 / EPS) * Y.T
            ya[p, D] = -1.0 / EPS
            ya[p, D + 1] = -yy / EPS
            yb[p, :D] = Y.T
            yb[p, D] = yy
            yb[p, D + 1] = 1.0
            xb[p, :D] = (2.0 / EPS) * X.T
            xb[p, D] = -1.0 / EPS
            xb[p, D + 1] = -xx / EPS
        in_maps.append({"xa": xa, "ya": ya, "xb": xb, "yb": yb})
    return in_maps


def host_finish(results):
    Lk = np.zeros((B, K), dtype=np.float64)
    for c in range(NCORES):
        part = np.asarray(results[c]["out"], dtype=np.float64)  # [PAIRS, NT]
        for p in range(PAIRS):
            g = c * PAIRS + p
            Lk[g // K, g % K] = EPS * part[p].sum()
    z = -Lk / TAU
    m = z.max(axis=1, keepdims=True)
    lse = m[:, 0] + np.log(np.exp(z - m).sum(axis=1))
    loss = -TAU * lse.mean()
    return np.float32(loss)


_CACHE = {}


def _get_program():
    if "nc" not in _CACHE:
        _CACHE["nc"] = build_program()
    return _CACHE["nc"]


def kernel(sim_seq, expert, starts):
    nc = _get_program()
    in_maps = host_prep(sim_seq, expert, starts)
    res = run_bass_kernel_spmd(nc, in_maps, list(range(NCORES)))
    return host_finish(res.results)


if __name__ == "__main__":
    import reference as ref

    inputs = ref.setup_inputs()
    expected = np.asarray(ref.reference(**inputs))
    actual = kernel(**{k: np.asarray(v) for k, v in inputs.items()})
    rel = abs(float(actual) - float(expected)) / abs(float(expected))
    print("expected:", expected, "actual:", actual, "rel err:", rel)
